# revision 25
# baseline (speedup 1.0000x reference)
"""BERT self-attention kernel for Trainium2, 8-core SPMD. v4.

Problem: hidden_states [S=2048, B=2, H=1024], 16 heads x 64, fp32.
Sharding: core i handles batch b = i//4 and head-group hg = i%4
(4 heads = 256 contiguous columns of Wq/Wk/Wv).

Design:
  - Host transposes hs -> hsT [H, S] bf16 (no PE transposes on chip),
    pre-scales Wk by G so scores arrive in the exp-approx domain, and
    post-processes the output (divide by the sumexp row, transpose).
  - On chip per core:
      qT/kT = W.T @ hsT      [128(d, 2 heads), S] bf16 per head-pair
      v     = hsT.T @ Wv     [t, 256] f32 psum -> fp8 (+ hi/lo residual
                             for the first NLO key-chunks) + ones col
      scT   = kT_h.T @ qT_h  [t, s] quadrant-packed pairs (K=64 at PE
                             rows 0/64) -> psum [128, 2, 512] f32
      expT  = exp-ish(scT)   one engine per (t, sb) unit, pattern-
                             balanced: ScalarE table exp / DVE custom op
                             EXP8 ((x+A)((x+B)^2+C))^8 ~ exp(x/(8G));
                             both write fp8e4 directly
      ctxT  = fp8 DoubleRow matmul over key-chunk PAIRS (contraction
              256 keys/MM): lhsT = [v(2j) | v(2j+1)], rhs =
              [expT(2j) | expT(2j+1)]; plus NLO/2 residual MMs with
              v_lo; accumulates [65, 512] f32 psum (row 64 = sumexp)
      out   = ctxT_aug [4 heads, 65, S] f32 DMA'd out; host divides.
  - Softmax normalization is free on chip: any per-head uniform scale
    of the exp approximation cancels in ctx/sumexp on the host.
"""

import numpy as np

S = 2048
B = 2
H = 1024
NH = 16
HD = 64
P = 128
HG = 256          # head-group width (4 heads) per core
SBLK = 512        # query block
NB = S // SBLK    # 4
NTCH = S // P     # 16 key chunks
KO = H // P       # 8 contraction chunks for projections
N_CORES = 8
NLO = 0           # key-chunks getting the v_lo fp8 residual correction

# exp approximation constants (see module docstring); fitted for
# scores in [-30, 30]:  ((u+A)((u+B)^2+C))^8 ~ exp(u/(8*G)) for u = s*G
EXP_A = 0.89989191
EXP_B = 0.39660346
EXP_C = 0.95369252
EXP_G = 1.0 / 116.722622
EXP_KAPPA = 14.59032776

_CACHE = {}


def _ref_exp8(in0, in1, s0, s1, imm2):
    x = in0.astype(np.float32)
    p = (x + s0) * ((x + s1) ** 2 + imm2)
    return ((p ** 2) ** 2) ** 2


def _register_exp8():
    import concourse.dve_ops as dve_ops
    from concourse.dve_spec import Spec, Src0, C0, C1, C2, sq, lower
    from concourse.dve_uop import DveOpSpec

    for op in dve_ops.OPS:
        if op.name == "EXP8_ANT":
            return op
    spec = Spec(
        body=sq(sq(sq((Src0 + C0) * (sq(Src0 + C1) + C2)))),
        reference=_ref_exp8,
    )
    opcode = dve_ops._CUSTOM_DVE_ROW_BASE + len(dve_ops.OPS)
    shas = {}
    for ver in ("v3", "v4"):
        try:
            s = DveOpSpec(name="EXP8_ANT", opcode=opcode,
                          uops=lower(spec, ver=ver), rd1_en=False)
            shas[ver] = s.sha(ver)
        except Exception:
            if ver == "v3":
                raise
    op = dve_ops.DveOp("EXP8_ANT", spec, subdim=False, uops_sha=shas)
    dve_ops.OPS.append(op)
    dve_ops.CUSTOM_DVE_SPECS[op.name] = op.spec
    dve_ops._SUB_OPCODE_FOR_NAME[op.name] = opcode
    return op


def _build_nc():
    import concourse.mybir as mybir
    import concourse.tile as tile
    from concourse import bacc

    exp8 = _register_exp8()

    f32 = mybir.dt.float32
    bf16 = mybir.dt.bfloat16
    fp8 = mybir.dt.float8e4
    Exp = mybir.ActivationFunctionType.Exp
    DR = mybir.MatmulPerfMode.DoubleRow

    nc = bacc.Bacc(None, target_bir_lowering=False)

    hsT_d = nc.dram_tensor("hsT", [H, S], bf16, kind="ExternalInput")
    wq_d = nc.dram_tensor("wq", [H, HG], bf16, kind="ExternalInput")
    wks_d = nc.dram_tensor("wks", [H, HG], bf16, kind="ExternalInput")
    wv_d = nc.dram_tensor("wv", [H, HG], bf16, kind="ExternalInput")
    out_d = nc.dram_tensor("outT", [4, HD + 1, S], f32, kind="ExternalOutput")

    with tile.TileContext(nc) as tc:
        with (
            tc.tile_pool(name="cst", bufs=1) as cst,
            tc.tile_pool(name="qkv", bufs=1) as qkv,
            tc.tile_pool(name="expp", bufs=2) as expp,
            tc.tile_pool(name="osb", bufs=2) as osb,
            tc.tile_pool(name="scp", bufs=1, space="PSUM") as scp,
            tc.tile_pool(name="cxp", bufs=1, space="PSUM") as cxp,
        ):
            # ---- static SBUF tensors --------------------------------
            hsT = cst.tile([P, KO, S], bf16)
            hs_v = hsT_d.rearrange("(ko p) s -> p ko s", p=P)
            w_sb = {}

            def _w_load(name, wd):
                w_sb[name] = cst.tile([P, KO, HG], bf16, name=f"w{name}")
                nc.sync.dma_start(
                    w_sb[name][:], wd.rearrange("(ko p) m -> p ko m", p=P)
                )

            # DMA priority order: wk pair-0 half, s-quarter-0 hs pieces,
            # then the rest.
            w_sb["k"] = cst.tile([P, KO, HG], bf16, name="wk")
            wk_v = wks_d.rearrange("(ko p) m -> p ko m", p=P)
            nc.sync.dma_start(w_sb["k"][:, :, 0:P], wk_v[:, :, 0:P])
            for ko in range(KO):
                nc.sync.dma_start(hsT[:, ko, 0:SBLK], hs_v[:, ko, 0:SBLK])
            _w_load("v", wv_d)
            _w_load("q", wq_d)
            for ko in range(KO):
                nc.sync.dma_start(hsT[:, ko, SBLK:2 * SBLK],
                                  hs_v[:, ko, SBLK:2 * SBLK])
            nc.sync.dma_start(w_sb["k"][:, :, P:HG], wk_v[:, :, P:HG])
            for sq_i in range(2, 4):
                for ko in range(KO):
                    nc.sync.dma_start(
                        hsT[:, ko, sq_i * SBLK:(sq_i + 1) * SBLK],
                        hs_v[:, ko, sq_i * SBLK:(sq_i + 1) * SBLK],
                    )

            # HAM warmup: ~80 junk matmuls keep the PE clock gate at
            # 8/8 while the input DMA streams in (PE is otherwise idle
            # until ~11.5us and its first 3.4us of real work runs cold).
            wrm = cst.tile([P, P], bf16, name="wrm")
            nc.gpsimd.memset(wrm[:], 0.0)
            wps = scp.tile([P, 2, SBLK], f32, tag="sc2",
                           name="wps")[:, 0, 0:P]
            for _ in range(40):
                nc.tensor.matmul(wps, wrm[:], wrm[:], start=True, stop=True)

            kT = [qkv.tile([P, S], bf16, tag=f"kT{p_}", name=f"kT{p_}")
                  for p_ in range(2)]
            qT = [qkv.tile([P, S], bf16, tag=f"qT{p_}", name=f"qT{p_}")
                  for p_ in range(2)]
            # v fp8: [t-in-chunk, chunk, head, 80] (65 used, padded so the
            # chunk (k-tile) stride is 320 B, a multiple of 16)
            v8 = qkv.tile([P, NTCH, 4, 80], fp8, tag="v8", name="v8")
            nc.gpsimd.memset(v8[:, :, :, HD:HD + 1], 1.0)
            if NLO:
                v8lo = qkv.tile([P, NLO, 4, 80], fp8, tag="v8lo", name="v8lo")
                nc.gpsimd.memset(v8lo[:, :, :, HD:HD + 1], 0.0)

            # ---- projections ----------------------------------------
            def qk_proj(which, pair, si, dst, eng, tag=None, halves=(0, 1),
                        _state={}):
                key = (which, pair, si)
                if 0 in halves:
                    _state[key] = scp.tile(
                        [P, 2, SBLK], f32, tag=tag or f"sc{si % 3}",
                        name="qk_ps")[:, 0, :]
                pst = _state[key]
                los = [0, 4] if halves == (0, 1) else [4 * halves[0]]
                for lo in los:
                    for ko in range(lo, lo + 4):
                        nc.tensor.matmul(
                            pst,
                            w_sb[which][:, ko, pair * P:(pair + 1) * P],
                            hsT[:, ko, si * SBLK:(si + 1) * SBLK],
                            start=(ko == 0), stop=(ko == KO - 1),
                        )
                if 1 in halves:
                    del _state[key]
                    if eng == 0:
                        nc.scalar.copy(dst, pst)
                    else:
                        nc.vector.tensor_copy(dst, pst)

            def v_proj(t, eng=1, tag=None, halves=(0, 1), _state={}):
                if 0 in halves:
                    _state[t] = scp.tile(
                        [P, 2, SBLK], f32, tag=tag or f"sc{t % 3}",
                        name="v_ps")[:, 0, 0:HG]
                pst = _state[t]
                los = [0, 4] if halves == (0, 1) else [4 * halves[0]]
                for lo in los:
                    for ko in range(lo, lo + 4):
                        nc.tensor.matmul(
                            pst,
                            hsT[:, ko, t * P:(t + 1) * P],
                            w_sb["v"][:, ko, :],
                            start=(ko == 0), stop=(ko == KO - 1),
                        )
                if 1 not in halves:
                    return
                del _state[t]
                pv = pst.rearrange("p (h d) -> p h d", d=HD)
                if eng == 0:
                    nc.scalar.copy(v8[:, t, :, 0:HD], pv)
                else:
                    nc.vector.tensor_copy(v8[:, t, :, 0:HD], pv)
                if NLO and t < NLO:
                    nc.vector.tensor_tensor(
                        v8lo[:, t, :, 0:HD], pv, v8[:, t, :, 0:HD],
                        mybir.AluOpType.subtract,
                    )

            # prologue: only what group (pair0, sb0) needs up front
            # prologue emission tracks DMA arrival: s-quarter q gates
            # k0[q], q0s0 and v chunks 4q..4q+3 (t-chunk t needs quarter
            # t//4); interleave so the in-order PE queue never idles.
            qk_proj("k", 0, 0, kT[0][:, 0:SBLK], 0)
            for t in (0, 1, 2, 3):
                v_proj(t, eng=t % 2)
            qk_proj("q", 0, 0, qT[0][:, 0:SBLK], 1)
            qk_proj("k", 0, 1, kT[0][:, SBLK:2 * SBLK], 1)
            for t in (4, 5, 6, 7):
                v_proj(t, eng=t % 2)
            qk_proj("k", 0, 2, kT[0][:, 2 * SBLK:3 * SBLK], 0)
            for t in (8, 9):
                v_proj(t, eng=t % 2)
            qk_proj("k", 0, 3, kT[0][:, 3 * SBLK:4 * SBLK], 1)

            # remaining projection units, woven into attention groups as
            # two 4-matmul half-chains at consecutive slots, psum tag
            # matched to the hook slot so the sc rotation is not disturbed
            def _half(fn, h):
                return lambda slot: fn(slot, h)

            def _qk_halves(which, pair, si, eng):
                dst = (kT if which == "k" else qT)[pair][
                    :, si * SBLK:(si + 1) * SBLK]
                return [
                    lambda slot, gi: qk_proj(
                        which, pair, si, dst, eng,
                        tag=f"sc{(slot + 2) % 3}", halves=(0,)),
                    lambda slot, gi: qk_proj(which, pair, si, dst, eng,
                                             tag=None, halves=(1,)),
                ]

            def _v_halves(t, eng):
                return [
                    lambda slot, gi: v_proj(
                        t, eng, tag=f"sc{(slot + 2) % 3}", halves=(0,)),
                    lambda slot, gi: v_proj(t, eng, tag=None, halves=(1,)),
                ]

            def _sched(units, slots):
                out = []
                for u, s0 in zip(units, slots):
                    h0, h1 = u
                    out += [(s0, h0), (s0 + 1, h1)]
                return out

            # weave[gi]: group gi = (pair gi//4, sb gi%4).  Constraints:
            # v8..15 inside group 0 before their pv_pair; q0[sb] before
            # group sb; k1 before group 4; q1[sb] before group 4+sb.
            weave = {
                0: _sched([_v_halves(10, 0), _v_halves(11, 1),
                           _v_halves(12, 0), _v_halves(13, 1),
                           _v_halves(14, 0), _v_halves(15, 1),
                           _qk_halves("q", 0, 1, 1)],
                          [1, 3, 5, 7, 9, 11, 13]),
                1: _sched([_qk_halves("q", 0, 2, 1),
                           _qk_halves("k", 1, 0, 0),
                           _qk_halves("k", 1, 1, 1)],
                          [2, 7, 12]),
                2: _sched([_qk_halves("q", 0, 3, 0),
                           _qk_halves("k", 1, 2, 1),
                           _qk_halves("k", 1, 3, 0)],
                          [2, 7, 12]),
                3: _sched([_qk_halves("q", 1, 0, 1),
                           _qk_halves("q", 1, 1, 0)],
                          [3, 9]),
                4: _sched([_qk_halves("q", 1, 2, 1)], [3]),
                5: _sched([_qk_halves("q", 1, 3, 0)], [3]),
            }

            # ---- attention ------------------------------------------
            # single-sb groups; sc triple-buffered to keep the PE queue
            # deep (hides the ~173 ns SBUF access latency per matmul);
            # exp alternates engines by t parity; leftover projection
            # units are woven in where the group has PE slack.
            def attention_group(pair, sb, gi):
                expt = expp.tile([P, NTCH, 2, SBLK], fp8,
                                 tag=f"e{gi % 2}", name=f"e{pair}{sb}")
                ctxps = [cxp.tile([HD + 1, SBLK], f32, tag=f"cx{h2}",
                                  name=f"cx{sb}{h2}") for h2 in range(2)]

                def scores_exp(t):
                    sc = scp.tile([P, 2, SBLK], f32, tag=f"sc{t % 3}",
                                  name=f"sc{t % 3}")
                    for h2 in range(2):
                        po = HD * h2
                        nc.tensor.matmul(
                            sc[:, h2, :],
                            kT[pair][po:po + HD, t * P:(t + 1) * P],
                            qT[pair][po:po + HD, sb * SBLK:(sb + 1) * SBLK],
                            start=True, stop=True,
                            tile_position=(po, 0),
                        )
                    # DVE takes odd t minus one per 16 (~47% of units)
                    use_dve = (t % 2 == 1) and not (t == 15 and gi % 2 == 0)
                    if use_dve:
                        nc.vector._custom_dve(
                            exp8, out=expt[:, t, :, :], in0=sc[:],
                            s0=EXP_A, s1=EXP_B, imm2=EXP_C,
                        )
                    else:
                        nc.scalar.activation(
                            expt[:, t, :, :], sc[:], Exp,
                            scale=EXP_KAPPA,
                        )

                def pv_pair(j, only_h2=None):
                    last = (j == NTCH // 2 - 1)
                    for h2 in range(2):
                        if only_h2 is not None and h2 != only_h2:
                            continue
                        head = pair * 2 + h2
                        nc.tensor.matmul(
                            ctxps[h2][:],
                            v8[:, 2 * j:2 * j + 2, head, 0:HD + 1],
                            expt[:, 2 * j:2 * j + 2, h2, :],
                            start=(j == 0),
                            stop=(last and not (NLO and 2 * j < NLO)),
                            perf_mode=DR,
                            skip_group_check=True,
                        )
                        if NLO and 2 * j < NLO:
                            nc.tensor.matmul(
                                ctxps[h2][:],
                                v8lo[:, 2 * j:2 * j + 2, head, 0:HD + 1],
                                expt[:, 2 * j:2 * j + 2, h2, :],
                                start=False, stop=last,
                                perf_mode=DR,
                                skip_group_check=True,
                            )

                hooks = {}
                for slot, fn in weave.get(gi, []):
                    hooks.setdefault(slot, []).append(fn)
                for t in range(NTCH):
                    scores_exp(t)
                    if t >= 5 and t % 2 == 1:
                        pv_pair((t - 5) // 2)
                    for fn in hooks.get(t, []):
                        fn(t, gi)
                # tail ordered by head so each head's output copy can
                # start as soon as its own chain stops
                for h2 in range(2):
                    for j in (NTCH // 2 - 2, NTCH // 2 - 1):
                        pv_pair(j, only_h2=h2)

                for h2 in range(2):
                    head = pair * 2 + h2
                    ot = osb.tile([HD + 1, SBLK], f32, tag="ot", name="ot")
                    if h2 == 0:
                        nc.scalar.copy(ot[:], ctxps[h2][:])
                    else:
                        nc.vector.tensor_copy(ot[:], ctxps[h2][:])
                    for c in range(2):
                        nc.sync.dma_start(
                            out_d[head, :,
                                  sb * SBLK + c * 256:sb * SBLK + (c + 1) * 256],
                            ot[:, c * 256:(c + 1) * 256],
                        )

            gi = 0
            for pair in range(2):
                for sb in range(NB):
                    attention_group(pair, sb, gi)
                    gi += 1

    nc.compile()
    return nc


def _get_nc():
    if "nc" not in _CACHE:
        _CACHE["nc"] = _build_nc()
    return _CACHE["nc"]


def _kernel_np(hidden_states, attention_mask, Wq, bq, Wk, bk, Wv, bv):
    """Numpy fallback for the general (mask/bias) case."""
    S_, B_, H_ = hidden_states.shape
    hd = H_ // NH

    def split(x):
        return x.reshape(S_, B_ * NH, hd).transpose(1, 0, 2)

    q = split(hidden_states @ Wq + bq)
    k = split(hidden_states @ Wk + bk)
    v = split(hidden_states @ Wv + bv)
    scores = np.einsum("nsd,ntd->nst", q, k).reshape(B_, NH, S_, S_)
    scores = scores / np.sqrt(np.float32(hd)) + attention_mask
    scores = scores - scores.max(axis=-1, keepdims=True)
    e = np.exp(scores)
    probs = (e / e.sum(axis=-1, keepdims=True)).reshape(B_ * NH, S_, S_)
    ctx = np.einsum("nst,ntd->nsd", probs.astype(np.float32), v)
    return ctx.transpose(1, 0, 2).reshape(S_, B_, H_).astype(np.float32)


def kernel(hidden_states, attention_mask, Wq, bq, Wk, bk, Wv, bv,
           _trace=False, _tmpdir=None):
    import ml_dtypes
    bf = ml_dtypes.bfloat16
    hidden_states = np.ascontiguousarray(hidden_states, dtype=np.float32)
    if (attention_mask is not None and np.any(attention_mask)) or \
            np.any(bq) or np.any(bk) or np.any(bv):
        return _kernel_np(hidden_states, attention_mask, Wq, bq, Wk, bk,
                          Wv, bv)

    from concourse.bass_utils import run_bass_kernel_spmd

    nc = _get_nc()
    # host-side prep
    hsT_b = [np.ascontiguousarray(hidden_states[:, b, :].T).astype(bf)
             for b in range(B)]
    wq_bf = np.asarray(Wq, np.float32).astype(bf)
    wks_bf = (np.asarray(Wk, np.float32) * EXP_G).astype(bf)
    wv_bf = np.asarray(Wv, np.float32).astype(bf)
    in_maps = []
    for core in range(N_CORES):
        b = core // 4
        hg = core % 4
        c0 = hg * HG
        in_maps.append({
            "hsT": hsT_b[b],
            "wq": np.ascontiguousarray(wq_bf[:, c0:c0 + HG]),
            "wks": np.ascontiguousarray(wks_bf[:, c0:c0 + HG]),
            "wv": np.ascontiguousarray(wv_bf[:, c0:c0 + HG]),
        })
    res = None
    last_err = None
    for _attempt in range(3):
        try:
            res = run_bass_kernel_spmd(
                nc, in_maps, core_ids=list(range(N_CORES)), trace=_trace,
                tmpdir=_tmpdir,
            )
            break
        except Exception as e:  # transient NRT/device hiccups: retry
            last_err = e
            import time as _time
            _time.sleep(2.0)
    if res is None:
        raise last_err
    out = np.empty((S, B, H), np.float32)
    for core in range(N_CORES):
        b = core // 4
        hg = core % 4
        r = res.results[core]["outT"]           # [4, 65, S]
        ctx = r[:, 0:HD, :] / r[:, HD:HD + 1, :]  # [4, 64, S]
        out[:, b, hg * HG:(hg + 1) * HG] = (
            ctx.transpose(2, 0, 1).reshape(S, HG)
        )
    if _trace:
        _CACHE["last_results"] = res
    return out


# revision 26
# speedup vs baseline: 1.0060x; 1.0060x over previous
"""BERT self-attention kernel for Trainium2, 8-core SPMD. v4.

Problem: hidden_states [S=2048, B=2, H=1024], 16 heads x 64, fp32.
Sharding: core i handles batch b = i//4 and head-group hg = i%4
(4 heads = 256 contiguous columns of Wq/Wk/Wv).

Design:
  - Host transposes hs -> hsT [H, S] bf16 (no PE transposes on chip),
    pre-scales Wk by G so scores arrive in the exp-approx domain, and
    post-processes the output (divide by the sumexp row, transpose).
  - On chip per core:
      qT/kT = W.T @ hsT      [128(d, 2 heads), S] bf16 per head-pair
      v     = hsT.T @ Wv     [t, 256] f32 psum -> fp8 (+ hi/lo residual
                             for the first NLO key-chunks) + ones col
      scT   = kT_h.T @ qT_h  [t, s] quadrant-packed pairs (K=64 at PE
                             rows 0/64) -> psum [128, 2, 512] f32
      expT  = exp-ish(scT)   one engine per (t, sb) unit, pattern-
                             balanced: ScalarE table exp / DVE custom op
                             EXP8 ((x+A)((x+B)^2+C))^8 ~ exp(x/(8G));
                             both write fp8e4 directly
      ctxT  = fp8 DoubleRow matmul over key-chunk PAIRS (contraction
              256 keys/MM): lhsT = [v(2j) | v(2j+1)], rhs =
              [expT(2j) | expT(2j+1)]; plus NLO/2 residual MMs with
              v_lo; accumulates [65, 512] f32 psum (row 64 = sumexp)
      out   = ctxT_aug [4 heads, 65, S] f32 DMA'd out; host divides.
  - Softmax normalization is free on chip: any per-head uniform scale
    of the exp approximation cancels in ctx/sumexp on the host.
"""

import numpy as np

S = 2048
B = 2
H = 1024
NH = 16
HD = 64
P = 128
HG = 256          # head-group width (4 heads) per core
SBLK = 512        # query block
NB = S // SBLK    # 4
NTCH = S // P     # 16 key chunks
KO = H // P       # 8 contraction chunks for projections
N_CORES = 8
NLO = 0           # key-chunks getting the v_lo fp8 residual correction

# exp approximation constants (see module docstring); fitted for
# scores in [-30, 30]:  ((u+A)((u+B)^2+C))^8 ~ exp(u/(8*G)) for u = s*G
EXP_A = 0.89989191
EXP_B = 0.39660346
EXP_C = 0.95369252
EXP_G = 1.0 / 116.722622
EXP_KAPPA = 14.59032776

_CACHE = {}


def _ref_exp8(in0, in1, s0, s1, imm2):
    x = in0.astype(np.float32)
    p = (x + s0) * ((x + s1) ** 2 + imm2)
    return ((p ** 2) ** 2) ** 2


def _register_exp8():
    import concourse.dve_ops as dve_ops
    from concourse.dve_spec import Spec, Src0, C0, C1, C2, sq, lower
    from concourse.dve_uop import DveOpSpec

    for op in dve_ops.OPS:
        if op.name == "EXP8_ANT":
            return op
    spec = Spec(
        body=sq(sq(sq((Src0 + C0) * (sq(Src0 + C1) + C2)))),
        reference=_ref_exp8,
    )
    opcode = dve_ops._CUSTOM_DVE_ROW_BASE + len(dve_ops.OPS)
    shas = {}
    for ver in ("v3", "v4"):
        try:
            s = DveOpSpec(name="EXP8_ANT", opcode=opcode,
                          uops=lower(spec, ver=ver), rd1_en=False)
            shas[ver] = s.sha(ver)
        except Exception:
            if ver == "v3":
                raise
    op = dve_ops.DveOp("EXP8_ANT", spec, subdim=False, uops_sha=shas)
    dve_ops.OPS.append(op)
    dve_ops.CUSTOM_DVE_SPECS[op.name] = op.spec
    dve_ops._SUB_OPCODE_FOR_NAME[op.name] = opcode
    return op


def _build_nc():
    import concourse.mybir as mybir
    import concourse.tile as tile
    from concourse import bacc

    exp8 = _register_exp8()

    f32 = mybir.dt.float32
    bf16 = mybir.dt.bfloat16
    fp8 = mybir.dt.float8e4
    Exp = mybir.ActivationFunctionType.Exp
    DR = mybir.MatmulPerfMode.DoubleRow

    nc = bacc.Bacc(None, target_bir_lowering=False)

    hsT_d = nc.dram_tensor("hsT", [H, S], bf16, kind="ExternalInput")
    wq_d = nc.dram_tensor("wq", [H, HG], bf16, kind="ExternalInput")
    wks_d = nc.dram_tensor("wks", [H, HG], bf16, kind="ExternalInput")
    wv_d = nc.dram_tensor("wv", [H, HG], bf16, kind="ExternalInput")
    out_d = nc.dram_tensor("outT", [4, HD + 1, S], f32, kind="ExternalOutput")

    with tile.TileContext(nc) as tc:
        with (
            tc.tile_pool(name="cst", bufs=1) as cst,
            tc.tile_pool(name="qkv", bufs=1) as qkv,
            tc.tile_pool(name="expp", bufs=2) as expp,
            tc.tile_pool(name="osb", bufs=2) as osb,
            tc.tile_pool(name="scp", bufs=1, space="PSUM") as scp,
            tc.tile_pool(name="cxp", bufs=1, space="PSUM") as cxp,
        ):
            # ---- static SBUF tensors --------------------------------
            hsT = cst.tile([P, KO, S], bf16)
            hs_v = hsT_d.rearrange("(ko p) s -> p ko s", p=P)
            w_sb = {}

            def _w_load(name, wd):
                w_sb[name] = cst.tile([P, KO, HG], bf16, name=f"w{name}")
                nc.sync.dma_start(
                    w_sb[name][:], wd.rearrange("(ko p) m -> p ko m", p=P)
                )

            # DMA priority order: wk pair-0 half, s-quarter-0 hs pieces,
            # then the rest.
            w_sb["k"] = cst.tile([P, KO, HG], bf16, name="wk")
            wk_v = wks_d.rearrange("(ko p) m -> p ko m", p=P)
            nc.sync.dma_start(w_sb["k"][:, :, 0:P], wk_v[:, :, 0:P])
            for ko in range(KO):
                nc.sync.dma_start(hsT[:, ko, 0:SBLK], hs_v[:, ko, 0:SBLK])
            _w_load("v", wv_d)
            _w_load("q", wq_d)
            for ko in range(KO):
                nc.sync.dma_start(hsT[:, ko, SBLK:2 * SBLK],
                                  hs_v[:, ko, SBLK:2 * SBLK])
            nc.sync.dma_start(w_sb["k"][:, :, P:HG], wk_v[:, :, P:HG])
            for sq_i in range(2, 4):
                for ko in range(KO):
                    nc.sync.dma_start(
                        hsT[:, ko, sq_i * SBLK:(sq_i + 1) * SBLK],
                        hs_v[:, ko, sq_i * SBLK:(sq_i + 1) * SBLK],
                    )

            # HAM warmup: ~80 junk matmuls keep the PE clock gate at
            # 8/8 while the input DMA streams in (PE is otherwise idle
            # until ~11.5us and its first 3.4us of real work runs cold).
            wrm = cst.tile([P, P], bf16, name="wrm")
            nc.gpsimd.memset(wrm[:], 0.0)
            wps = scp.tile([P, 2, SBLK], f32, tag="sc2",
                           name="wps")[:, 0, 0:P]
            for _ in range(40):
                nc.tensor.matmul(wps, wrm[:], wrm[:], start=True, stop=True)

            kT = [qkv.tile([P, S], bf16, tag=f"kT{p_}", name=f"kT{p_}")
                  for p_ in range(2)]
            qT = [qkv.tile([P, S], bf16, tag=f"qT{p_}", name=f"qT{p_}")
                  for p_ in range(2)]
            # v fp8: [t-in-chunk, chunk, head, 80] (65 used, padded so the
            # chunk (k-tile) stride is 320 B, a multiple of 16)
            v8 = qkv.tile([P, NTCH, 4, 80], fp8, tag="v8", name="v8")
            nc.gpsimd.memset(v8[:, :, :, HD:HD + 1], 1.0)
            if NLO:
                v8lo = qkv.tile([P, NLO, 4, 80], fp8, tag="v8lo", name="v8lo")
                nc.gpsimd.memset(v8lo[:, :, :, HD:HD + 1], 0.0)

            # ---- projections ----------------------------------------
            def qk_proj(which, pair, si, dst, eng, tag=None, halves=(0, 1),
                        _state={}):
                key = (which, pair, si)
                if 0 in halves:
                    _state[key] = scp.tile(
                        [P, 2, SBLK], f32, tag=tag or f"sc{si % 3}",
                        name="qk_ps")[:, 0, :]
                pst = _state[key]
                los = [0, 4] if halves == (0, 1) else [4 * halves[0]]
                for lo in los:
                    for ko in range(lo, lo + 4):
                        nc.tensor.matmul(
                            pst,
                            w_sb[which][:, ko, pair * P:(pair + 1) * P],
                            hsT[:, ko, si * SBLK:(si + 1) * SBLK],
                            start=(ko == 0), stop=(ko == KO - 1),
                        )
                if 1 in halves:
                    del _state[key]
                    if eng == 0:
                        nc.scalar.copy(dst, pst)
                    else:
                        nc.vector.tensor_copy(dst, pst)

            def v_proj(t, eng=1, tag=None, halves=(0, 1), _state={}):
                if 0 in halves:
                    _state[t] = scp.tile(
                        [P, 2, SBLK], f32, tag=tag or f"sc{t % 3}",
                        name="v_ps")[:, 0, 0:HG]
                pst = _state[t]
                los = [0, 4] if halves == (0, 1) else [4 * halves[0]]
                for lo in los:
                    for ko in range(lo, lo + 4):
                        nc.tensor.matmul(
                            pst,
                            hsT[:, ko, t * P:(t + 1) * P],
                            w_sb["v"][:, ko, :],
                            start=(ko == 0), stop=(ko == KO - 1),
                        )
                if 1 not in halves:
                    return
                del _state[t]
                pv = pst.rearrange("p (h d) -> p h d", d=HD)
                if eng == 0:
                    nc.scalar.copy(v8[:, t, :, 0:HD], pv)
                else:
                    nc.vector.tensor_copy(v8[:, t, :, 0:HD], pv)
                if NLO and t < NLO:
                    nc.vector.tensor_tensor(
                        v8lo[:, t, :, 0:HD], pv, v8[:, t, :, 0:HD],
                        mybir.AluOpType.subtract,
                    )

            # prologue: only what group (pair0, sb0) needs up front
            # prologue emission tracks DMA arrival: s-quarter q gates
            # k0[q], q0s0 and v chunks 4q..4q+3 (t-chunk t needs quarter
            # t//4); interleave so the in-order PE queue never idles.
            qk_proj("k", 0, 0, kT[0][:, 0:SBLK], 0)
            for t in (0, 1, 2, 3):
                v_proj(t, eng=t % 2)
            qk_proj("q", 0, 0, qT[0][:, 0:SBLK], 1)
            qk_proj("k", 0, 1, kT[0][:, SBLK:2 * SBLK], 1)
            for t in (4, 5, 6, 7):
                v_proj(t, eng=t % 2)
            qk_proj("k", 0, 2, kT[0][:, 2 * SBLK:3 * SBLK], 0)
            for t in (8, 9):
                v_proj(t, eng=t % 2)
            qk_proj("k", 0, 3, kT[0][:, 3 * SBLK:4 * SBLK], 1)

            # remaining projection units, woven into attention groups as
            # two 4-matmul half-chains at consecutive slots, psum tag
            # matched to the hook slot so the sc rotation is not disturbed
            def _half(fn, h):
                return lambda slot: fn(slot, h)

            def _qk_halves(which, pair, si, eng):
                dst = (kT if which == "k" else qT)[pair][
                    :, si * SBLK:(si + 1) * SBLK]
                return [
                    lambda slot, gi: qk_proj(
                        which, pair, si, dst, eng,
                        tag=f"sc{(slot + 2) % 3}", halves=(0,)),
                    lambda slot, gi: qk_proj(which, pair, si, dst, eng,
                                             tag=None, halves=(1,)),
                ]

            def _v_halves(t, eng):
                return [
                    lambda slot, gi: v_proj(
                        t, eng, tag=f"sc{(slot + 2) % 3}", halves=(0,)),
                    lambda slot, gi: v_proj(t, eng, tag=None, halves=(1,)),
                ]

            def _sched(units, slots):
                out = []
                for u, s0 in zip(units, slots):
                    h0, h1 = u
                    out += [(s0, h0), (s0 + 1, h1)]
                return out

            # weave[gi]: group gi = (pair gi//4, sb gi%4).  Constraints:
            # v8..15 inside group 0 before their pv_pair; q0[sb] before
            # group sb; k1 before group 4; q1[sb] before group 4+sb.
            weave = {
                0: _sched([_v_halves(10, 0), _v_halves(11, 1),
                           _v_halves(12, 0), _v_halves(13, 1),
                           _v_halves(14, 0), _v_halves(15, 1),
                           _qk_halves("q", 0, 1, 1)],
                          [1, 3, 5, 7, 9, 11, 13]),
                1: _sched([_qk_halves("q", 0, 2, 1),
                           _qk_halves("k", 1, 0, 0),
                           _qk_halves("k", 1, 1, 1)],
                          [2, 7, 12]),
                2: _sched([_qk_halves("q", 0, 3, 0),
                           _qk_halves("k", 1, 2, 1),
                           _qk_halves("k", 1, 3, 0)],
                          [2, 7, 12]),
                3: _sched([_qk_halves("q", 1, 0, 1),
                           _qk_halves("q", 1, 1, 0)],
                          [3, 9]),
                4: _sched([_qk_halves("q", 1, 2, 1)], [3]),
                5: _sched([_qk_halves("q", 1, 3, 0)], [3]),
            }

            # ---- attention ------------------------------------------
            # single-sb groups; sc triple-buffered to keep the PE queue
            # deep (hides the ~173 ns SBUF access latency per matmul);
            # exp alternates engines by t parity; leftover projection
            # units are woven in where the group has PE slack.
            def attention_group(pair, sb, gi):
                expt = expp.tile([P, NTCH, 2, SBLK], fp8,
                                 tag=f"e{gi % 2}", name=f"e{pair}{sb}")
                ctxps = [cxp.tile([HD + 1, SBLK], f32, tag=f"cx{h2}",
                                  name=f"cx{sb}{h2}") for h2 in range(2)]

                def scores_exp(t):
                    sc = scp.tile([P, 2, SBLK], f32, tag=f"sc{t % 3}",
                                  name=f"sc{t % 3}")
                    for h2 in range(2):
                        po = HD * h2
                        nc.tensor.matmul(
                            sc[:, h2, :],
                            kT[pair][po:po + HD, t * P:(t + 1) * P],
                            qT[pair][po:po + HD, sb * SBLK:(sb + 1) * SBLK],
                            start=True, stop=True,
                            tile_position=(po, 0),
                        )
                    # DVE takes odd t minus one per 16 (~47% of units)
                    use_dve = (t % 2 == 1) and not (t == 15 and gi % 2 == 0)
                    if use_dve:
                        nc.vector._custom_dve(
                            exp8, out=expt[:, t, :, :], in0=sc[:],
                            s0=EXP_A, s1=EXP_B, imm2=EXP_C,
                        )
                    else:
                        nc.scalar.activation(
                            expt[:, t, :, :], sc[:], Exp,
                            scale=EXP_KAPPA,
                        )

                def pv_pair(j):
                    last = (j == NTCH // 2 - 1)
                    for h2 in range(2):
                        head = pair * 2 + h2
                        nc.tensor.matmul(
                            ctxps[h2][:],
                            v8[:, 2 * j:2 * j + 2, head, 0:HD + 1],
                            expt[:, 2 * j:2 * j + 2, h2, :],
                            start=(j == 0),
                            stop=(last and not (NLO and 2 * j < NLO)),
                            perf_mode=DR,
                            skip_group_check=True,
                        )
                        if NLO and 2 * j < NLO:
                            nc.tensor.matmul(
                                ctxps[h2][:],
                                v8lo[:, 2 * j:2 * j + 2, head, 0:HD + 1],
                                expt[:, 2 * j:2 * j + 2, h2, :],
                                start=False, stop=last,
                                perf_mode=DR,
                                skip_group_check=True,
                            )

                hooks = {}
                for slot, fn in weave.get(gi, []):
                    hooks.setdefault(slot, []).append(fn)
                for t in range(NTCH):
                    scores_exp(t)
                    if t >= 5 and t % 2 == 1:
                        pv_pair((t - 5) // 2)
                    for fn in hooks.get(t, []):
                        fn(t, gi)
                for j in (NTCH // 2 - 2, NTCH // 2 - 1):
                    pv_pair(j)

                for h2 in range(2):
                    head = pair * 2 + h2
                    ot = osb.tile([HD + 1, SBLK], f32, tag="ot", name="ot")
                    if h2 == 0:
                        nc.scalar.copy(ot[:], ctxps[h2][:])
                    else:
                        nc.vector.tensor_copy(ot[:], ctxps[h2][:])
                    for c in range(2):
                        nc.sync.dma_start(
                            out_d[head, :,
                                  sb * SBLK + c * 256:sb * SBLK + (c + 1) * 256],
                            ot[:, c * 256:(c + 1) * 256],
                        )

            gi = 0
            for pair in range(2):
                for sb in range(NB):
                    attention_group(pair, sb, gi)
                    gi += 1

    nc.compile()
    return nc


def _get_nc():
    if "nc" not in _CACHE:
        _CACHE["nc"] = _build_nc()
    return _CACHE["nc"]


def _kernel_np(hidden_states, attention_mask, Wq, bq, Wk, bk, Wv, bv):
    """Numpy fallback for the general (mask/bias) case."""
    S_, B_, H_ = hidden_states.shape
    hd = H_ // NH

    def split(x):
        return x.reshape(S_, B_ * NH, hd).transpose(1, 0, 2)

    q = split(hidden_states @ Wq + bq)
    k = split(hidden_states @ Wk + bk)
    v = split(hidden_states @ Wv + bv)
    scores = np.einsum("nsd,ntd->nst", q, k).reshape(B_, NH, S_, S_)
    scores = scores / np.sqrt(np.float32(hd)) + attention_mask
    scores = scores - scores.max(axis=-1, keepdims=True)
    e = np.exp(scores)
    probs = (e / e.sum(axis=-1, keepdims=True)).reshape(B_ * NH, S_, S_)
    ctx = np.einsum("nst,ntd->nsd", probs.astype(np.float32), v)
    return ctx.transpose(1, 0, 2).reshape(S_, B_, H_).astype(np.float32)


def kernel(hidden_states, attention_mask, Wq, bq, Wk, bk, Wv, bv,
           _trace=False, _tmpdir=None):
    import ml_dtypes
    bf = ml_dtypes.bfloat16
    hidden_states = np.ascontiguousarray(hidden_states, dtype=np.float32)
    if (attention_mask is not None and np.any(attention_mask)) or \
            np.any(bq) or np.any(bk) or np.any(bv):
        return _kernel_np(hidden_states, attention_mask, Wq, bq, Wk, bk,
                          Wv, bv)

    from concourse.bass_utils import run_bass_kernel_spmd

    nc = _get_nc()
    # host-side prep
    hsT_b = [np.ascontiguousarray(hidden_states[:, b, :].T).astype(bf)
             for b in range(B)]
    wq_bf = np.asarray(Wq, np.float32).astype(bf)
    wks_bf = (np.asarray(Wk, np.float32) * EXP_G).astype(bf)
    wv_bf = np.asarray(Wv, np.float32).astype(bf)
    in_maps = []
    for core in range(N_CORES):
        b = core // 4
        hg = core % 4
        c0 = hg * HG
        in_maps.append({
            "hsT": hsT_b[b],
            "wq": np.ascontiguousarray(wq_bf[:, c0:c0 + HG]),
            "wks": np.ascontiguousarray(wks_bf[:, c0:c0 + HG]),
            "wv": np.ascontiguousarray(wv_bf[:, c0:c0 + HG]),
        })
    res = None
    last_err = None
    for _attempt in range(3):
        try:
            res = run_bass_kernel_spmd(
                nc, in_maps, core_ids=list(range(N_CORES)), trace=_trace,
                tmpdir=_tmpdir,
            )
            break
        except Exception as e:  # transient NRT/device hiccups: retry
            last_err = e
            import time as _time
            _time.sleep(2.0)
    if res is None:
        raise last_err
    out = np.empty((S, B, H), np.float32)
    for core in range(N_CORES):
        b = core // 4
        hg = core % 4
        r = res.results[core]["outT"]           # [4, 65, S]
        ctx = r[:, 0:HD, :] / r[:, HD:HD + 1, :]  # [4, 64, S]
        out[:, b, hg * HG:(hg + 1) * HG] = (
            ctx.transpose(2, 0, 1).reshape(S, HG)
        )
    if _trace:
        _CACHE["last_results"] = res
    return out


# revision 27
# speedup vs baseline: 1.0158x; 1.0098x over previous
"""BERT self-attention kernel for Trainium2, 8-core SPMD. v4.

Problem: hidden_states [S=2048, B=2, H=1024], 16 heads x 64, fp32.
Sharding: core i handles batch b = i//4 and head-group hg = i%4
(4 heads = 256 contiguous columns of Wq/Wk/Wv).

Design:
  - Host transposes hs -> hsT [H, S] bf16 (no PE transposes on chip),
    pre-scales Wk by G so scores arrive in the exp-approx domain, and
    post-processes the output (divide by the sumexp row, transpose).
  - On chip per core:
      qT/kT = W.T @ hsT      [128(d, 2 heads), S] bf16 per head-pair
      v     = hsT.T @ Wv     [t, 256] f32 psum -> fp8 (+ hi/lo residual
                             for the first NLO key-chunks) + ones col
      scT   = kT_h.T @ qT_h  [t, s] quadrant-packed pairs (K=64 at PE
                             rows 0/64) -> psum [128, 2, 512] f32
      expT  = exp-ish(scT)   one engine per (t, sb) unit, pattern-
                             balanced: ScalarE table exp / DVE custom op
                             EXP8 ((x+A)((x+B)^2+C))^8 ~ exp(x/(8G));
                             both write fp8e4 directly
      ctxT  = fp8 DoubleRow matmul over key-chunk PAIRS (contraction
              256 keys/MM): lhsT = [v(2j) | v(2j+1)], rhs =
              [expT(2j) | expT(2j+1)]; plus NLO/2 residual MMs with
              v_lo; accumulates [65, 512] f32 psum (row 64 = sumexp)
      out   = ctxT_aug [4 heads, 65, S] f32 DMA'd out; host divides.
  - Softmax normalization is free on chip: any per-head uniform scale
    of the exp approximation cancels in ctx/sumexp on the host.
"""

import numpy as np

S = 2048
B = 2
H = 1024
NH = 16
HD = 64
P = 128
HG = 256          # head-group width (4 heads) per core
SBLK = 512        # query block
NB = S // SBLK    # 4
NTCH = S // P     # 16 key chunks
KO = H // P       # 8 contraction chunks for projections
N_CORES = 8
NLO = 0           # key-chunks getting the v_lo fp8 residual correction

# exp approximation constants (see module docstring); fitted for
# scores in [-30, 30]:  ((u+A)((u+B)^2+C))^8 ~ exp(u/(8*G)) for u = s*G
EXP_A = 0.89989191
EXP_B = 0.39660346
EXP_C = 0.95369252
EXP_G = 1.0 / 116.722622
EXP_KAPPA = 14.59032776

_CACHE = {}


def _ref_exp8(in0, in1, s0, s1, imm2):
    x = in0.astype(np.float32)
    p = (x + s0) * ((x + s1) ** 2 + imm2)
    return ((p ** 2) ** 2) ** 2


def _register_exp8():
    import concourse.dve_ops as dve_ops
    from concourse.dve_spec import Spec, Src0, C0, C1, C2, sq, lower
    from concourse.dve_uop import DveOpSpec

    for op in dve_ops.OPS:
        if op.name == "EXP8_ANT":
            return op
    spec = Spec(
        body=sq(sq(sq((Src0 + C0) * (sq(Src0 + C1) + C2)))),
        reference=_ref_exp8,
    )
    opcode = dve_ops._CUSTOM_DVE_ROW_BASE + len(dve_ops.OPS)
    shas = {}
    for ver in ("v3", "v4"):
        try:
            s = DveOpSpec(name="EXP8_ANT", opcode=opcode,
                          uops=lower(spec, ver=ver), rd1_en=False)
            shas[ver] = s.sha(ver)
        except Exception:
            if ver == "v3":
                raise
    op = dve_ops.DveOp("EXP8_ANT", spec, subdim=False, uops_sha=shas)
    dve_ops.OPS.append(op)
    dve_ops.CUSTOM_DVE_SPECS[op.name] = op.spec
    dve_ops._SUB_OPCODE_FOR_NAME[op.name] = opcode
    return op


def _build_nc():
    import concourse.mybir as mybir
    import concourse.tile as tile
    from concourse import bacc

    exp8 = _register_exp8()

    f32 = mybir.dt.float32
    bf16 = mybir.dt.bfloat16
    fp8 = mybir.dt.float8e4
    Exp = mybir.ActivationFunctionType.Exp
    DR = mybir.MatmulPerfMode.DoubleRow

    nc = bacc.Bacc(None, target_bir_lowering=False)

    hsT_d = nc.dram_tensor("hsT", [H, S], bf16, kind="ExternalInput")
    wq_d = nc.dram_tensor("wq", [H, HG], bf16, kind="ExternalInput")
    wks_d = nc.dram_tensor("wks", [H, HG], bf16, kind="ExternalInput")
    wv_d = nc.dram_tensor("wv", [H, HG], bf16, kind="ExternalInput")
    out_d = nc.dram_tensor("outT", [4, HD + 1, S], f32, kind="ExternalOutput")

    with tile.TileContext(nc) as tc:
        with (
            tc.tile_pool(name="cst", bufs=1) as cst,
            tc.tile_pool(name="qkv", bufs=1) as qkv,
            tc.tile_pool(name="expp", bufs=2) as expp,
            tc.tile_pool(name="osb", bufs=2) as osb,
            tc.tile_pool(name="scp", bufs=1, space="PSUM") as scp,
            tc.tile_pool(name="cxp", bufs=1, space="PSUM") as cxp,
        ):
            # ---- static SBUF tensors --------------------------------
            hsT = cst.tile([P, KO, S], bf16)
            hs_v = hsT_d.rearrange("(ko p) s -> p ko s", p=P)
            w_sb = {}

            def _w_load(name, wd):
                w_sb[name] = cst.tile([P, KO, HG], bf16, name=f"w{name}")
                nc.sync.dma_start(
                    w_sb[name][:], wd.rearrange("(ko p) m -> p ko m", p=P)
                )

            # DMA priority order: wk pair-0 half, s-quarter-0 hs pieces,
            # then the rest.
            w_sb["k"] = cst.tile([P, KO, HG], bf16, name="wk")
            wk_v = wks_d.rearrange("(ko p) m -> p ko m", p=P)
            nc.sync.dma_start(w_sb["k"][:, :, 0:P], wk_v[:, :, 0:P])
            for ko in range(KO):
                nc.sync.dma_start(hsT[:, ko, 0:SBLK], hs_v[:, ko, 0:SBLK])
            _w_load("v", wv_d)
            _w_load("q", wq_d)
            for ko in range(KO):
                nc.sync.dma_start(hsT[:, ko, SBLK:2 * SBLK],
                                  hs_v[:, ko, SBLK:2 * SBLK])
            nc.sync.dma_start(w_sb["k"][:, :, P:HG], wk_v[:, :, P:HG])
            for sq_i in range(2, 4):
                for ko in range(KO):
                    nc.sync.dma_start(
                        hsT[:, ko, sq_i * SBLK:(sq_i + 1) * SBLK],
                        hs_v[:, ko, sq_i * SBLK:(sq_i + 1) * SBLK],
                    )

            # HAM warmup: ~80 junk matmuls keep the PE clock gate at
            # 8/8 while the input DMA streams in (PE is otherwise idle
            # until ~11.5us and its first 3.4us of real work runs cold).
            wrm = cst.tile([P, P], bf16, name="wrm")
            nc.gpsimd.memset(wrm[:], 0.0)
            wps = scp.tile([P, 2, SBLK], f32, tag="sc2",
                           name="wps")[:, 0, 0:P]
            for _ in range(40):
                nc.tensor.matmul(wps, wrm[:], wrm[:], start=True, stop=True)

            kT = [qkv.tile([P, S], bf16, tag=f"kT{p_}", name=f"kT{p_}")
                  for p_ in range(2)]
            qT = [qkv.tile([P, S], bf16, tag=f"qT{p_}", name=f"qT{p_}")
                  for p_ in range(2)]
            # v fp8: [t-in-chunk, chunk, head, 80] (65 used, padded so the
            # chunk (k-tile) stride is 320 B, a multiple of 16)
            v8 = qkv.tile([P, NTCH, 4, 80], fp8, tag="v8", name="v8")
            nc.gpsimd.memset(v8[:, :, :, HD:HD + 1], 1.0)
            if NLO:
                v8lo = qkv.tile([P, NLO, 4, 80], fp8, tag="v8lo", name="v8lo")
                nc.gpsimd.memset(v8lo[:, :, :, HD:HD + 1], 0.0)

            # ---- projections ----------------------------------------
            def qk_proj(which, pair, si, dst, eng, tag=None, halves=(0, 1),
                        _state={}):
                key = (which, pair, si)
                if 0 in halves:
                    _state[key] = scp.tile(
                        [P, 2, SBLK], f32, tag=tag or f"sc{si % 3}",
                        name="qk_ps")[:, 0, :]
                pst = _state[key]
                los = [0, 4] if halves == (0, 1) else [4 * halves[0]]
                for lo in los:
                    for ko in range(lo, lo + 4):
                        nc.tensor.matmul(
                            pst,
                            w_sb[which][:, ko, pair * P:(pair + 1) * P],
                            hsT[:, ko, si * SBLK:(si + 1) * SBLK],
                            start=(ko == 0), stop=(ko == KO - 1),
                        )
                if 1 in halves:
                    del _state[key]
                    if eng == 0:
                        nc.scalar.copy(dst, pst)
                    else:
                        nc.vector.tensor_copy(dst, pst)

            def v_proj(t, eng=1, tag=None, halves=(0, 1), _state={}):
                if 0 in halves:
                    _state[t] = scp.tile(
                        [P, 2, SBLK], f32, tag=tag or f"sc{t % 3}",
                        name="v_ps")[:, 0, 0:HG]
                pst = _state[t]
                los = [0, 4] if halves == (0, 1) else [4 * halves[0]]
                for lo in los:
                    for ko in range(lo, lo + 4):
                        nc.tensor.matmul(
                            pst,
                            hsT[:, ko, t * P:(t + 1) * P],
                            w_sb["v"][:, ko, :],
                            start=(ko == 0), stop=(ko == KO - 1),
                        )
                if 1 not in halves:
                    return
                del _state[t]
                pv = pst.rearrange("p (h d) -> p h d", d=HD)
                if eng == 0:
                    nc.scalar.copy(v8[:, t, :, 0:HD], pv)
                else:
                    nc.vector.tensor_copy(v8[:, t, :, 0:HD], pv)
                if NLO and t < NLO:
                    nc.vector.tensor_tensor(
                        v8lo[:, t, :, 0:HD], pv, v8[:, t, :, 0:HD],
                        mybir.AluOpType.subtract,
                    )

            # prologue: only what group (pair0, sb0) needs up front
            # prologue emission tracks DMA arrival: s-quarter q gates
            # k0[q], q0s0 and v chunks 4q..4q+3 (t-chunk t needs quarter
            # t//4); interleave so the in-order PE queue never idles.
            qk_proj("k", 0, 0, kT[0][:, 0:SBLK], 0)
            for t in (0, 1, 2, 3):
                v_proj(t, eng=t % 2)
            qk_proj("q", 0, 0, qT[0][:, 0:SBLK], 1)
            qk_proj("k", 0, 1, kT[0][:, SBLK:2 * SBLK], 1)
            for t in (4, 5, 6, 7):
                v_proj(t, eng=t % 2)
            qk_proj("k", 0, 2, kT[0][:, 2 * SBLK:3 * SBLK], 0)
            for t in (8, 9):
                v_proj(t, eng=t % 2)
            qk_proj("k", 0, 3, kT[0][:, 3 * SBLK:4 * SBLK], 1)

            # remaining projection units, woven into attention groups as
            # two 4-matmul half-chains at consecutive slots, psum tag
            # matched to the hook slot so the sc rotation is not disturbed
            def _half(fn, h):
                return lambda slot: fn(slot, h)

            def _qk_halves(which, pair, si, eng):
                dst = (kT if which == "k" else qT)[pair][
                    :, si * SBLK:(si + 1) * SBLK]
                return [
                    lambda slot, gi: qk_proj(
                        which, pair, si, dst, eng,
                        tag=f"sc{(slot + 2) % 3}", halves=(0,)),
                    lambda slot, gi: qk_proj(which, pair, si, dst, eng,
                                             tag=None, halves=(1,)),
                ]

            def _v_halves(t, eng):
                return [
                    lambda slot, gi: v_proj(
                        t, eng, tag=f"sc{(slot + 2) % 3}", halves=(0,)),
                    lambda slot, gi: v_proj(t, eng, tag=None, halves=(1,)),
                ]

            def _sched(units, slots):
                out = []
                for u, s0 in zip(units, slots):
                    h0, h1 = u
                    out += [(s0, h0), (s0 + 1, h1)]
                return out

            # weave[gi]: group gi = (pair gi//4, sb gi%4).  Constraints:
            # v8..15 inside group 0 before their pv_pair; q0[sb] before
            # group sb; k1 before group 4; q1[sb] before group 4+sb.
            weave = {
                0: _sched([_v_halves(10, 0), _v_halves(11, 1),
                           _v_halves(12, 0), _v_halves(13, 1),
                           _v_halves(14, 0), _v_halves(15, 1),
                           _qk_halves("q", 0, 1, 1)],
                          [1, 3, 5, 7, 9, 11, 13]),
                1: _sched([_qk_halves("q", 0, 2, 1),
                           _qk_halves("k", 1, 0, 0),
                           _qk_halves("k", 1, 1, 1)],
                          [2, 7, 12]),
                2: _sched([_qk_halves("q", 0, 3, 0),
                           _qk_halves("k", 1, 2, 1),
                           _qk_halves("k", 1, 3, 0)],
                          [2, 7, 12]),
                3: _sched([_qk_halves("q", 1, 0, 1),
                           _qk_halves("q", 1, 1, 0)],
                          [3, 9]),
                4: _sched([_qk_halves("q", 1, 2, 1)], [3]),
                5: _sched([_qk_halves("q", 1, 3, 0)], [3]),
            }

            # ---- attention ------------------------------------------
            # single-sb groups; sc triple-buffered to keep the PE queue
            # deep (hides the ~173 ns SBUF access latency per matmul);
            # exp alternates engines by t parity; leftover projection
            # units are woven in where the group has PE slack.
            def attention_group(pair, sb, gi):
                expt = expp.tile([P, NTCH, 2, SBLK], fp8,
                                 tag=f"e{gi % 2}", name=f"e{pair}{sb}")
                ctxps = [cxp.tile([HD + 1, SBLK], f32, tag=f"cx{h2}",
                                  name=f"cx{sb}{h2}") for h2 in range(2)]

                def scores_exp(t):
                    sc = scp.tile([P, 2, SBLK], f32, tag=f"sc{t % 3}",
                                  name=f"sc{t % 3}")
                    for h2 in range(2):
                        po = HD * h2
                        nc.tensor.matmul(
                            sc[:, h2, :],
                            kT[pair][po:po + HD, t * P:(t + 1) * P],
                            qT[pair][po:po + HD, sb * SBLK:(sb + 1) * SBLK],
                            start=True, stop=True,
                            tile_position=(po, 0),
                        )
                    # DVE takes odd t minus one per 16 (~47% of units)
                    use_dve = (t % 2 == 1) and not (t == 15 and gi % 2 == 0)
                    if use_dve:
                        nc.vector._custom_dve(
                            exp8, out=expt[:, t, :, :], in0=sc[:],
                            s0=EXP_A, s1=EXP_B, imm2=EXP_C,
                        )
                    else:
                        nc.scalar.activation(
                            expt[:, t, :, :], sc[:], Exp,
                            scale=EXP_KAPPA,
                        )

                def pv_pair(j):
                    last = (j == NTCH // 2 - 1)
                    for h2 in range(2):
                        head = pair * 2 + h2
                        nc.tensor.matmul(
                            ctxps[h2][:],
                            v8[:, 2 * j:2 * j + 2, head, 0:HD + 1],
                            expt[:, 2 * j:2 * j + 2, h2, :],
                            start=(j == 0),
                            stop=(last and not (NLO and 2 * j < NLO)),
                            perf_mode=DR,
                            skip_group_check=True,
                        )
                        if NLO and 2 * j < NLO:
                            nc.tensor.matmul(
                                ctxps[h2][:],
                                v8lo[:, 2 * j:2 * j + 2, head, 0:HD + 1],
                                expt[:, 2 * j:2 * j + 2, h2, :],
                                start=False, stop=last,
                                perf_mode=DR,
                                skip_group_check=True,
                            )

                hooks = {}
                for slot, fn in weave.get(gi, []):
                    hooks.setdefault(slot, []).append(fn)
                for t in range(NTCH):
                    scores_exp(t)
                    if t >= 5 and t % 2 == 1:
                        pv_pair((t - 5) // 2)
                    for fn in hooks.get(t, []):
                        fn(t, gi)
                for j in (NTCH // 2 - 2, NTCH // 2 - 1):
                    pv_pair(j)

                for h2 in range(2):
                    head = pair * 2 + h2
                    ot = osb.tile([HD + 1, SBLK], f32, tag="ot", name="ot")
                    if h2 == 0:
                        nc.scalar.copy(ot[:], ctxps[h2][:])
                    else:
                        nc.vector.tensor_copy(ot[:], ctxps[h2][:])
                    nc.sync.dma_start(
                        out_d[head, :, sb * SBLK:(sb + 1) * SBLK], ot[:]
                    )

            gi = 0
            for pair in range(2):
                for sb in range(NB):
                    attention_group(pair, sb, gi)
                    gi += 1

    nc.compile()
    return nc


def _get_nc():
    if "nc" not in _CACHE:
        _CACHE["nc"] = _build_nc()
    return _CACHE["nc"]


def _kernel_np(hidden_states, attention_mask, Wq, bq, Wk, bk, Wv, bv):
    """Numpy fallback for the general (mask/bias) case."""
    S_, B_, H_ = hidden_states.shape
    hd = H_ // NH

    def split(x):
        return x.reshape(S_, B_ * NH, hd).transpose(1, 0, 2)

    q = split(hidden_states @ Wq + bq)
    k = split(hidden_states @ Wk + bk)
    v = split(hidden_states @ Wv + bv)
    scores = np.einsum("nsd,ntd->nst", q, k).reshape(B_, NH, S_, S_)
    scores = scores / np.sqrt(np.float32(hd)) + attention_mask
    scores = scores - scores.max(axis=-1, keepdims=True)
    e = np.exp(scores)
    probs = (e / e.sum(axis=-1, keepdims=True)).reshape(B_ * NH, S_, S_)
    ctx = np.einsum("nst,ntd->nsd", probs.astype(np.float32), v)
    return ctx.transpose(1, 0, 2).reshape(S_, B_, H_).astype(np.float32)


def kernel(hidden_states, attention_mask, Wq, bq, Wk, bk, Wv, bv,
           _trace=False, _tmpdir=None):
    import ml_dtypes
    bf = ml_dtypes.bfloat16
    hidden_states = np.ascontiguousarray(hidden_states, dtype=np.float32)
    if (attention_mask is not None and np.any(attention_mask)) or \
            np.any(bq) or np.any(bk) or np.any(bv):
        return _kernel_np(hidden_states, attention_mask, Wq, bq, Wk, bk,
                          Wv, bv)

    from concourse.bass_utils import run_bass_kernel_spmd

    nc = _get_nc()
    # host-side prep
    hsT_b = [np.ascontiguousarray(hidden_states[:, b, :].T).astype(bf)
             for b in range(B)]
    wq_bf = np.asarray(Wq, np.float32).astype(bf)
    wks_bf = (np.asarray(Wk, np.float32) * EXP_G).astype(bf)
    wv_bf = np.asarray(Wv, np.float32).astype(bf)
    in_maps = []
    for core in range(N_CORES):
        b = core // 4
        hg = core % 4
        c0 = hg * HG
        in_maps.append({
            "hsT": hsT_b[b],
            "wq": np.ascontiguousarray(wq_bf[:, c0:c0 + HG]),
            "wks": np.ascontiguousarray(wks_bf[:, c0:c0 + HG]),
            "wv": np.ascontiguousarray(wv_bf[:, c0:c0 + HG]),
        })
    res = None
    last_err = None
    for _attempt in range(3):
        try:
            res = run_bass_kernel_spmd(
                nc, in_maps, core_ids=list(range(N_CORES)), trace=_trace,
                tmpdir=_tmpdir,
            )
            break
        except Exception as e:  # transient NRT/device hiccups: retry
            last_err = e
            import time as _time
            _time.sleep(2.0)
    if res is None:
        raise last_err
    out = np.empty((S, B, H), np.float32)
    for core in range(N_CORES):
        b = core // 4
        hg = core % 4
        r = res.results[core]["outT"]           # [4, 65, S]
        ctx = r[:, 0:HD, :] / r[:, HD:HD + 1, :]  # [4, 64, S]
        out[:, b, hg * HG:(hg + 1) * HG] = (
            ctx.transpose(2, 0, 1).reshape(S, HG)
        )
    if _trace:
        _CACHE["last_results"] = res
    return out
